# revision 4
# baseline (speedup 1.0000x reference)
"""Trainium2 Bass kernel for nn_DirectOrderingModel (MLP + pairwise ordering loss).

Self-contained: hardcodes B=4096, H=1024, D=4, 8 cores.

Structure (two NEFF launches via run_bass_kernel_spmd):
  Kernel A (per core, 512-row shard, features-on-partitions):
      LN1 -> W1 -> gelu -> LN2 -> W2 -> gelu -> Wh -> sigmoid(tanh form)
      + per-core MSE partial.  Outputs logitsT [4,512], mse_col [4,1].
  Host: gathers logits, builds per-core loss windows (pure data movement).
  Kernel B (per core): pairwise loss over this core's 4 i-blocks using the
      cyclic half-strip decomposition (each unordered pair covered once).
      Zero ACT table switches: softplus/sigmoid computed via exp/ln only:
        spl = Ln(F_j * Finv_i + 1) = softplus(20(L_j - L_i))
        t   = Exp(-spl)            = sigma(20(L_i - L_j))
        sp  = Ln(E_j * Einv_i + 1) = softplus(20(P_j - P_i))
      gate m = (spl > softplus(.2)) | (spl < softplus(-.2))  (== |ldiff|>0.01)
      bce_masked_sum = sum m*sp + 20*(sum gP~ - sum Pi*racc~) with
        g~ = (t-1)*m = -m*(1-t);  racc~ = rowsums g~;  gP~ = sum g~*P_j.
  Host: combines per-core partials (float64) into the scalar loss.
"""
import os
import numpy as np

import concourse.bass as bass
import concourse.bacc as bacc
import concourse.mybir as mybir
import concourse.tile as tile
from concourse import bass_utils

F32 = mybir.dt.float32
BF16 = mybir.dt.bfloat16
AF = mybir.ActivationFunctionType
OP = mybir.AluOpType

B, H, D = 4096, 1024, 4
H2, H3 = 512, 256
NC = 8
R = B // NC                      # 512 rows per core
SCALE = 20.0
LN_EPS = 1e-5
W_UNIT = 2176                    # diag block (128) + 16 strip blocks (2048)
W_WIN = 2560                     # window width = 128*3 + W_UNIT
HI = np.float32(np.log1p(np.exp(0.2)))    # softplus(0.2)
LO = np.float32(np.log1p(np.exp(-0.2)))   # softplus(-0.2)

_cache = {}


def _build_mlp():
    nc = bacc.Bacc("TRN2", target_bir_lowering=False, debug=False, num_devices=NC)
    xT = nc.dram_tensor("xT", [H, R], F32, kind="ExternalInput")
    W1 = nc.dram_tensor("W1", [H, H2], F32, kind="ExternalInput")
    W2 = nc.dram_tensor("W2", [H2, H3], F32, kind="ExternalInput")
    Wh = nc.dram_tensor("Wh", [H3, D], F32, kind="ExternalInput")
    b1c = nc.dram_tensor("b1c", [128, H2 // 128], F32, kind="ExternalInput")
    b2c = nc.dram_tensor("b2c", [128, H3 // 128], F32, kind="ExternalInput")
    bh2 = nc.dram_tensor("bh2", [D, 1], F32, kind="ExternalInput")
    g1c = nc.dram_tensor("g1c", [128, H // 128], F32, kind="ExternalInput")
    b1lc = nc.dram_tensor("b1lc", [128, H // 128], F32, kind="ExternalInput")
    g2c = nc.dram_tensor("g2c", [128, H2 // 128], F32, kind="ExternalInput")
    b2lc = nc.dram_tensor("b2lc", [128, H2 // 128], F32, kind="ExternalInput")
    labshT = nc.dram_tensor("labshT", [D, R], F32, kind="ExternalInput")
    out_logitsT = nc.dram_tensor("out_logitsT", [D, R], F32, kind="ExternalOutput")
    out_mse = nc.dram_tensor("out_mse", [D, 1], F32, kind="ExternalOutput")

    with tile.TileContext(nc) as tc:
        with (
            tc.tile_pool(name="wpool", bufs=1) as wp,
            tc.tile_pool(name="apool", bufs=1) as ap_,
            tc.tile_pool(name="spool", bufs=2) as sp_,
            tc.tile_pool(name="pp", bufs=1, space="PSUM") as pp,
            tc.tile_pool(name="pp2", bufs=2, space="PSUM") as pp2,
        ):
            ones128 = wp.tile([128, 1], F32)
            nc.vector.memset(ones128[:, :], 1.0)
            ones1 = wp.tile([1, 128], F32)
            nc.vector.memset(ones1[:, :], 1.0)
            eps_t = wp.tile([1, 1], F32)
            nc.vector.memset(eps_t[:, :], LN_EPS)

            # ---- load weights & inputs
            w1t = [wp.tile([128, H2], F32, name=f"w1t{k}") for k in range(8)]
            for k in range(8):
                nc.sync.dma_start(w1t[k][:, :], W1[k * 128:(k + 1) * 128, :])
            w2t = [wp.tile([128, H3], F32, name=f"w2t{k}") for k in range(4)]
            for k in range(4):
                nc.sync.dma_start(w2t[k][:, :], W2[k * 128:(k + 1) * 128, :])
            wht = [wp.tile([128, D], F32, name=f"wht{k}") for k in range(2)]
            for k in range(2):
                nc.sync.dma_start(wht[k][:, :], Wh[k * 128:(k + 1) * 128, :])
            g1s = wp.tile([128, 8], F32); nc.sync.dma_start(g1s[:, :], g1c[:, :])
            b1ls = wp.tile([128, 8], F32); nc.sync.dma_start(b1ls[:, :], b1lc[:, :])
            g2s = wp.tile([128, 4], F32); nc.sync.dma_start(g2s[:, :], g2c[:, :])
            b2ls = wp.tile([128, 4], F32); nc.sync.dma_start(b2ls[:, :], b2lc[:, :])
            b1s = wp.tile([128, 4], F32); nc.sync.dma_start(b1s[:, :], b1c[:, :])
            b2s = wp.tile([128, 2], F32); nc.sync.dma_start(b2s[:, :], b2c[:, :])
            bh2s = wp.tile([D, 1], F32); nc.sync.dma_start(bh2s[:, :], bh2[:, :])

            xk = [ap_.tile([128, R], F32, name=f"xk{k}") for k in range(8)]
            for k in range(8):
                nc.sync.dma_start(xk[k][:, :], xT[k * 128:(k + 1) * 128, :])

            def layer_norm(chunks, n_feat, gs, bls, tag):
                """chunks: list of [128,R] SBUF tiles (features on partitions).
                Returns normalized chunks (new tiles)."""
                nk = len(chunks)
                mu_ps = pp.tile([1, R], F32, name=f"mu_ps_{tag}", tag="mups")
                ss_ps = pp.tile([1, R], F32, name=f"ss_ps_{tag}", tag="ssps")
                for k in range(nk):
                    sq = sp_.tile([128, R], F32, name=f"sq_{tag}", tag=f"sq_{tag}")
                    nc.scalar.activation(sq[:, :], chunks[k][:, :], AF.Square)
                    nc.tensor.matmul(mu_ps[:, :], ones128[:, :], chunks[k][:, :],
                                     start=(k == 0), stop=(k == nk - 1))
                    nc.tensor.matmul(ss_ps[:, :], ones128[:, :], sq[:, :],
                                     start=(k == 0), stop=(k == nk - 1))
                inv = 1.0 / n_feat
                mu = ap_.tile([1, R], F32, name=f"mu_{tag}")
                nc.vector.tensor_scalar(mu[:, :], mu_ps[:, :], inv, None, OP.mult)
                ms = ap_.tile([1, R], F32, name=f"ms_{tag}")
                nc.vector.tensor_scalar(ms[:, :], ss_ps[:, :], inv, None, OP.mult)
                mu2 = ap_.tile([1, R], F32, name=f"mu2_{tag}")
                nc.vector.tensor_tensor(mu2[:, :], mu[:, :], mu[:, :], OP.mult)
                var = ap_.tile([1, R], F32, name=f"var_{tag}")
                nc.vector.tensor_tensor(var[:, :], ms[:, :], mu2[:, :], OP.subtract)
                # rstd = exp(-0.5*ln(var+eps))   (exp/ln table set)
                lnv = ap_.tile([1, R], F32, name=f"lnv_{tag}")
                nc.scalar.activation(lnv[:, :], var[:, :], AF.Ln, bias=eps_t[0:1, 0:1], scale=1.0)
                rstd = ap_.tile([1, R], F32, name=f"rstd_{tag}")
                nc.scalar.activation(rstd[:, :], lnv[:, :], AF.Exp, bias=0.0, scale=-0.5)
                # broadcast mu, rstd to [128,R] in PSUM
                mu_b = pp.tile([128, R], F32, name=f"mu_b_{tag}", tag="mub")
                nc.tensor.matmul(mu_b[:, :], ones1[:, :], mu[:, :], start=True, stop=True)
                rstd_b = pp.tile([128, R], F32, name=f"rstd_b_{tag}", tag="rstdb")
                nc.tensor.matmul(rstd_b[:, :], ones1[:, :], rstd[:, :], start=True, stop=True)
                outs = []
                for k in range(nk):
                    u = sp_.tile([128, R], F32, name=f"u_{tag}", tag=f"u_{tag}")
                    nc.vector.tensor_tensor(u[:, :], chunks[k][:, :], mu_b[:, :], OP.subtract)
                    u2 = sp_.tile([128, R], F32, name=f"u2_{tag}", tag=f"u2_{tag}")
                    nc.vector.tensor_tensor(u2[:, :], u[:, :], rstd_b[:, :], OP.mult)
                    xn = ap_.tile([128, R], F32, name=f"xn_{tag}{k}")
                    nc.scalar.activation(xn[:, :], u2[:, :], AF.Identity,
                                         bias=bls[:, k:k + 1], scale=gs[:, k:k + 1])
                    outs.append(xn)
                return outs

            xn1 = layer_norm(xk, H, g1s, b1ls, "ln1")

            # ---- mm1 + gelu -> h1T chunks [128,R] x4
            h1 = []
            for m in range(4):
                h1_ps = pp2.tile([128, R], F32, name="h1_ps", tag="h1ps")
                for k in range(8):
                    nc.tensor.matmul(h1_ps[:, :], w1t[k][:, m * 128:(m + 1) * 128],
                                     xn1[k][:, :], start=(k == 0), stop=(k == 7))
                h1m = ap_.tile([128, R], F32, name=f"h1_{m}")
                nc.scalar.activation(h1m[:, :], h1_ps[:, :], AF.Gelu,
                                     bias=b1s[:, m:m + 1], scale=1.0)
                h1.append(h1m)

            xn2 = layer_norm(h1, H2, g2s, b2ls, "ln2")

            # ---- mm2 + gelu -> h2T chunks [128,R] x2
            h2 = []
            for m in range(2):
                h2_ps = pp2.tile([128, R], F32, name="h2_ps", tag="h1ps")
                for k in range(4):
                    nc.tensor.matmul(h2_ps[:, :], w2t[k][:, m * 128:(m + 1) * 128],
                                     xn2[k][:, :], start=(k == 0), stop=(k == 3))
                h2m = ap_.tile([128, R], F32, name=f"h2_{m}")
                nc.scalar.activation(h2m[:, :], h2_ps[:, :], AF.Gelu,
                                     bias=b2s[:, m:m + 1], scale=1.0)
                h2.append(h2m)

            # ---- head: logitsT = 0.5 + 0.5*tanh((h2T.T@Wh + bh)/2)
            lg_ps = pp.tile([D, R], F32, name="lg_ps", tag="mups")
            for k in range(2):
                nc.tensor.matmul(lg_ps[:, :], wht[k][:, :], h2[k][:, :],
                                 start=(k == 0), stop=(k == 1))
            th = ap_.tile([D, R], F32, name="th")
            nc.scalar.activation(th[:, :], lg_ps[:, :], AF.Tanh,
                                 bias=bh2s[:, :], scale=0.5)
            logitsT = ap_.tile([D, R], F32, name="logitsT")
            nc.vector.tensor_scalar(logitsT[:, :], th[:, :], 0.5, 0.5, OP.mult, OP.add)
            nc.sync.dma_start(out_logitsT[:, :], logitsT[:, :])

            # ---- mse partial: sum (logits-labels)^2 per d-partition
            labs = ap_.tile([D, R], F32, name="labs")
            nc.sync.dma_start(labs[:, :], labshT[:, :])
            d1 = ap_.tile([D, R], F32, name="d1")
            nc.vector.tensor_tensor(d1[:, :], logitsT[:, :], labs[:, :], OP.subtract)
            msec = ap_.tile([D, 1], F32, name="msec")
            scrm = ap_.tile([D, R], F32, name="scrm")
            nc.vector.scalar_tensor_tensor(scrm[:, :], d1[:, :], 1.0, d1[:, :],
                                           OP.mult, OP.mult, accum_out=msec[:, :])
            nc.sync.dma_start(out_mse[:, :], msec[:, :])
    nc.compile()
    return nc


def _build_loss():
    nc = bacc.Bacc("TRN2", target_bir_lowering=False, debug=False, num_devices=NC)
    Lwin = nc.dram_tensor("Lwin", [D, W_WIN], F32, kind="ExternalInput")
    Pwin = nc.dram_tensor("Pwin", [D, W_WIN], F32, kind="ExternalInput")
    Lcols = nc.dram_tensor("Lcols", [128, 16], F32, kind="ExternalInput")
    Pcols = nc.dram_tensor("Pcols", [128, 16], F32, kind="ExternalInput")
    maskin = nc.dram_tensor("maskin", [128, 256], BF16, kind="ExternalInput")
    out_stats = nc.dram_tensor("out_stats", [1, 8], F32, kind="ExternalOutput")

    with tile.TileContext(nc) as tc:
        with (
            tc.tile_pool(name="cst", bufs=1) as cst,
            tc.tile_pool(name="bc", bufs=2) as bc,
            tc.tile_pool(name="tp", bufs=2) as tp,
            tc.tile_pool(name="ps", bufs=1, space="PSUM") as ps,
            tc.tile_pool(name="dr", bufs=1, space="DRAM") as dr,
        ):
            ones_bf = cst.tile([128, 1], BF16)
            nc.vector.memset(ones_bf[:, :], 1.0)
            ones_f = cst.tile([128, 1], F32)
            nc.vector.memset(ones_f[:, :], 1.0)
            mask_sb = cst.tile([128, 256], BF16)
            nc.sync.dma_start(mask_sb[:, :], maskin[:, :])

            # one-time: exp rows (scoped pool so the SBUF is reclaimed)
            rows_F = dr.tile([D, W_WIN], F32)
            rows_E = dr.tile([D, W_WIN], F32)
            with tc.tile_pool(name="tmp1", bufs=1) as t1:
                lw = t1.tile([D, W_WIN], F32)
                nc.sync.dma_start(lw[:, :], Lwin[:, :])
                pw = t1.tile([D, W_WIN], F32)
                nc.sync.dma_start(pw[:, :], Pwin[:, :])
                fw = t1.tile([D, W_WIN], F32)
                nc.scalar.activation(fw[:, :], lw[:, :], AF.Exp, bias=0.0, scale=SCALE)
                ew = t1.tile([D, W_WIN], F32)
                nc.scalar.activation(ew[:, :], pw[:, :], AF.Exp, bias=0.0, scale=SCALE)
                nc.sync.dma_start(rows_F[:, :], fw[:, :])
                nc.sync.dma_start(rows_E[:, :], ew[:, :])

            lc = cst.tile([128, 16], F32)
            nc.sync.dma_start(lc[:, :], Lcols[:, :])
            pc = cst.tile([128, 16], F32)
            nc.sync.dma_start(pc[:, :], Pcols[:, :])
            finv = cst.tile([128, 16], F32)
            nc.scalar.activation(finv[:, :], lc[:, :], AF.Exp, bias=0.0, scale=-SCALE)
            einv = cst.tile([128, 16], F32)
            nc.scalar.activation(einv[:, :], pc[:, :], AF.Exp, bias=0.0, scale=-SCALE)

            racc = cst.tile([128, 16], F32)
            gpc = cst.tile([128, 16], F32)
            mspc = cst.tile([128, 16], F32)
            cnt_ps = ps.tile([1, 512], F32)

            first = [True]
            slices = [(0, 512), (512, 1024), (1024, 1536), (1536, 2048), (2048, 2176)]
            for d in range(D):
                Fb = bc.tile([128, W_WIN], F32, name="Fb", tag="Fb")
                nc.sync.dma_start(Fb[:, :], rows_F[d:d + 1, :].partition_broadcast(128))
                Eb = bc.tile([128, W_WIN], F32, name="Eb", tag="Eb")
                nc.sync.dma_start(Eb[:, :], rows_E[d:d + 1, :].partition_broadcast(128))
                Pb = bc.tile([128, W_WIN], F32, name="Pb", tag="Pb")
                nc.sync.dma_start(Pb[:, :], Pwin[d:d + 1, :].partition_broadcast(128))
                for s in range(4):
                    col = s * 4 + d
                    j0 = 128 * s
                    spl = tp.tile([128, W_UNIT], F32, name="spl", tag="spl")
                    nc.scalar.activation(spl[:, :], Fb[:, j0:j0 + W_UNIT], AF.Ln,
                                         bias=1.0, scale=finv[:, col:col + 1])
                    tt_ = tp.tile([128, W_UNIT], F32, name="tt_", tag="tt_")
                    nc.scalar.activation(tt_[:, :], spl[:, :], AF.Exp, bias=0.0, scale=-1.0)
                    sp = tp.tile([128, W_UNIT], F32, name="sp", tag="sp")
                    nc.scalar.activation(sp[:, :], Eb[:, j0:j0 + W_UNIT], AF.Ln,
                                         bias=1.0, scale=einv[:, col:col + 1])
                    m1 = tp.tile([128, W_UNIT], BF16, name="m1", tag="m1")
                    nc.gpsimd.tensor_scalar(m1[:, :], spl[:, :], float(HI), None, OP.is_gt)
                    m = tp.tile([128, W_UNIT], BF16, name="m", tag="m")
                    nc.vector.scalar_tensor_tensor(m[:, :], spl[:, :], float(LO), m1[:, :],
                                                   OP.is_lt, OP.add)
                    # diag + wrap masks
                    nc.vector.tensor_tensor(m[:, 0:128], m[:, 0:128],
                                            mask_sb[:, 0:128], OP.mult)
                    nc.vector.tensor_tensor(m[:, 2048:2176], m[:, 2048:2176],
                                            mask_sb[:, 128:256], OP.mult)
                    gt = tp.tile([128, W_UNIT], BF16, name="gt", tag="gt")
                    nc.vector.scalar_tensor_tensor(gt[:, :], tt_[:, :], 1.0, m[:, :],
                                                   OP.subtract, OP.mult,
                                                   accum_out=racc[:, col:col + 1])
                    scr = tp.tile([128, W_UNIT], BF16, name="scr", tag="scr")
                    nc.vector.scalar_tensor_tensor(scr[:, :], gt[:, :], 1.0,
                                                   Pb[:, j0:j0 + W_UNIT],
                                                   OP.mult, OP.mult,
                                                   accum_out=gpc[:, col:col + 1])
                    msp = tp.tile([128, W_UNIT], BF16, name="msp", tag="msp")
                    nc.vector.scalar_tensor_tensor(msp[:, :], sp[:, :], 1.0, m[:, :],
                                                   OP.mult, OP.mult,
                                                   accum_out=mspc[:, col:col + 1])
                    for (a, b_) in slices:
                        w = b_ - a
                        nc.tensor.matmul(cnt_ps[:, 0:w], ones_bf[:, :], m[:, a:b_],
                                         start=first[0], stop=False)
                        first[0] = False

            # ---- finals
            cnt_sb = cst.tile([1, 512], F32)
            nc.vector.tensor_copy(cnt_sb[:, :], cnt_ps[:, :])
            stats = cst.tile([1, 8], F32)
            nc.vector.memset(stats[:, :], 0.0)
            nc.vector.tensor_reduce(stats[0:1, 0:1], cnt_sb[:, :],
                                    mybir.AxisListType.X, OP.add)
            rpp = cst.tile([128, 16], F32)
            nc.vector.tensor_tensor(rpp[:, :], racc[:, :], pc[:, :], OP.mult)
            stack = cst.tile([128, 3], F32)
            nc.vector.tensor_reduce(stack[:, 0:1], rpp[:, :], mybir.AxisListType.X, OP.add)
            nc.vector.tensor_reduce(stack[:, 1:2], gpc[:, :], mybir.AxisListType.X, OP.add)
            nc.vector.tensor_reduce(stack[:, 2:3], mspc[:, :], mybir.AxisListType.X, OP.add)
            fin_ps = ps.tile([1, 4], F32)
            nc.tensor.matmul(fin_ps[:, 0:3], ones_f[:, :], stack[:, :], start=True, stop=True)
            nc.vector.tensor_copy(stats[0:1, 1:2], fin_ps[:, 2:3])
            nc.vector.tensor_copy(stats[0:1, 2:4], fin_ps[:, 0:2])
            nc.sync.dma_start(out_stats[:, :], stats[:, :])
    nc.compile()
    return nc


def _get(name):
    if name not in _cache:
        _cache[name] = _build_mlp() if name == "mlp" else _build_loss()
    return _cache[name]


def kernel(pooled, labels, ln1_g, ln1_b, W1, b1, ln2_g, ln2_b, W2, b2, Wh, bh):
    pooled = np.asarray(pooled, np.float32)
    labels = np.asarray(labels, np.float32)
    W1 = np.asarray(W1, np.float32); W2 = np.asarray(W2, np.float32)
    Wh = np.asarray(Wh, np.float32)

    # ---- kernel A: MLP ----
    pooledT = np.ascontiguousarray(pooled.T)              # [H, B]
    labelsT = np.ascontiguousarray(labels.T)              # [D, B]
    common = {
        "W1": W1, "W2": W2, "Wh": Wh,
        "b1c": np.ascontiguousarray(np.asarray(b1, np.float32).reshape(4, 128).T),
        "b2c": np.ascontiguousarray(np.asarray(b2, np.float32).reshape(2, 128).T),
        "bh2": np.asarray(bh, np.float32).reshape(D, 1) * 0.5,
        "g1c": np.ascontiguousarray(np.asarray(ln1_g, np.float32).reshape(8, 128).T),
        "b1lc": np.ascontiguousarray(np.asarray(ln1_b, np.float32).reshape(8, 128).T),
        "g2c": np.ascontiguousarray(np.asarray(ln2_g, np.float32).reshape(4, 128).T),
        "b2lc": np.ascontiguousarray(np.asarray(ln2_b, np.float32).reshape(4, 128).T),
    }
    in_maps_a = []
    for c in range(NC):
        in_maps_a.append(dict(
            common,
            xT=np.ascontiguousarray(pooledT[:, c * R:(c + 1) * R]),
            labshT=np.ascontiguousarray(labelsT[:, c * R:(c + 1) * R]),
        ))
    nc_a = _get("mlp")
    res_a = bass_utils.run_bass_kernel_spmd(
        nc_a, in_maps_a, core_ids=list(range(NC)),
        trace=bool(int(os.environ.get("KTRACE", "0"))))
    logits = np.concatenate(
        [res_a.results[c]["out_logitsT"].T for c in range(NC)], 0)  # [B, D]
    mse_ss = sum(float(res_a.results[c]["out_mse"].sum()) for c in range(NC))

    # ---- kernel B: pairwise loss ----
    logitsT = np.ascontiguousarray(logits.T)              # [D, B]
    Pext = np.concatenate([logitsT, logitsT[:, :W_WIN]], 1)
    Lext = np.concatenate([labelsT, labelsT[:, :W_WIN]], 1)
    tri = np.triu(np.ones((128, 128), np.float32), 1)
    low = 1.0 - tri                                        # lower incl diag
    import ml_dtypes
    in_maps_b = []
    for c in range(NC):
        wrap = tri if c < 4 else low
        maskin = np.concatenate([tri, wrap], 1).astype(ml_dtypes.bfloat16)
        pcols = np.empty((128, 16), np.float32)
        lcols = np.empty((128, 16), np.float32)
        for s in range(4):
            for d in range(D):
                pcols[:, s * 4 + d] = logits[c * R + 128 * s: c * R + 128 * (s + 1), d]
                lcols[:, s * 4 + d] = labels[c * R + 128 * s: c * R + 128 * (s + 1), d]
        in_maps_b.append({
            "Lwin": np.ascontiguousarray(Lext[:, c * R: c * R + W_WIN]),
            "Pwin": np.ascontiguousarray(Pext[:, c * R: c * R + W_WIN]),
            "Lcols": lcols, "Pcols": pcols, "maskin": maskin,
        })
    nc_b = _get("loss")
    res_b = bass_utils.run_bass_kernel_spmd(
        nc_b, in_maps_b, core_ids=list(range(NC)),
        trace=bool(int(os.environ.get("KTRACE", "0"))))

    cnt = 0.0; spsum = 0.0; rp = 0.0; gp = 0.0
    for c in range(NC):
        st = np.asarray(res_b.results[c]["out_stats"], np.float64)
        cnt += st[0, 0]; spsum += st[0, 1]; rp += st[0, 2]; gp += st[0, 3]

    ordering_sum = spsum + SCALE * (gp - rp)
    ordering = ordering_sum / cnt if cnt > 0 else 0.0
    mse = mse_ss / (B * D)
    loss = np.float32(0.5 * mse + 0.5 * ordering)
    kernel._last = (res_a, res_b)
    return np.array(loss, np.float32), logits


# revision 6
# speedup vs baseline: 2.4987x; 2.4987x over previous
"""Trainium2 Bass kernel for nn_DirectOrderingModel (MLP + pairwise ordering loss).

Self-contained: hardcodes B=4096, H=1024, D=4, 8 cores.

Structure (two NEFF launches via run_bass_kernel_spmd):
  Kernel A (per core, 512-row shard, features-on-partitions):
      LN1 -> W1 -> gelu -> LN2 -> W2 -> gelu -> Wh -> sigmoid(tanh form)
      + per-core MSE partial.  Outputs logitsT [4,512], mse_col [4,1].
  Host: gathers logits, builds per-core loss windows (pure data movement).
  Kernel B (per core): pairwise loss over this core's 4 i-blocks using the
      cyclic half-strip decomposition (each unordered pair covered once).
      Zero ACT table switches: softplus/sigmoid computed via exp/ln only:
        spl = Ln(F_j * Finv_i + 1) = softplus(20(L_j - L_i))
        t   = Exp(-spl)            = sigma(20(L_i - L_j))
        sp  = Ln(E_j * Einv_i + 1) = softplus(20(P_j - P_i))
      gate m = (spl > softplus(.2)) | (spl < softplus(-.2))  (== |ldiff|>0.01)
      bce_masked_sum = sum m*sp + 20*(sum gP~ - sum Pi*racc~) with
        g~ = (t-1)*m = -m*(1-t);  racc~ = rowsums g~;  gP~ = sum g~*P_j.
  Host: combines per-core partials (float64) into the scalar loss.
"""
import os
import numpy as np

import concourse.bass as bass
import concourse.bacc as bacc
import concourse.mybir as mybir
import concourse.tile as tile
from concourse import bass_utils

F32 = mybir.dt.float32
BF16 = mybir.dt.bfloat16
AF = mybir.ActivationFunctionType
OP = mybir.AluOpType

MLP_R = bool(int(os.environ.get("MLP_R", "0")))
B, H, D = 4096, 1024, 4
H2, H3 = 512, 256
NC = 8
R = B // NC                      # 512 rows per core
SCALE = 20.0
LN_EPS = 1e-5
W_UNIT = 2176                    # diag block (128) + 16 strip blocks (2048)
W_WIN = 2560                     # window width = 128*3 + W_UNIT
HI = np.float32(np.log1p(np.exp(0.2)))    # softplus(0.2)
LO = np.float32(np.log1p(np.exp(-0.2)))   # softplus(-0.2)

_cache = {}


def _r32(ap):
    return ap.bitcast(mybir.dt.float32r) if MLP_R else ap


def _build_mlp():
    nc = bacc.Bacc("TRN2", target_bir_lowering=False, debug=False, num_devices=NC)
    xT = nc.dram_tensor("xT", [H, R], F32, kind="ExternalInput")
    W1 = nc.dram_tensor("W1", [H, H2], F32, kind="ExternalInput")
    W2 = nc.dram_tensor("W2", [H2, H3], F32, kind="ExternalInput")
    Wh = nc.dram_tensor("Wh", [H3, D], F32, kind="ExternalInput")
    b1c = nc.dram_tensor("b1c", [128, H2 // 128], F32, kind="ExternalInput")
    b2c = nc.dram_tensor("b2c", [128, H3 // 128], F32, kind="ExternalInput")
    bh2 = nc.dram_tensor("bh2", [D, 1], F32, kind="ExternalInput")
    g1c = nc.dram_tensor("g1c", [128, H // 128], F32, kind="ExternalInput")
    b1lc = nc.dram_tensor("b1lc", [128, H // 128], F32, kind="ExternalInput")
    g2c = nc.dram_tensor("g2c", [128, H2 // 128], F32, kind="ExternalInput")
    b2lc = nc.dram_tensor("b2lc", [128, H2 // 128], F32, kind="ExternalInput")
    labshT = nc.dram_tensor("labshT", [D, R], F32, kind="ExternalInput")
    out_logitsT = nc.dram_tensor("out_logitsT", [D, R], F32, kind="ExternalOutput")
    out_mse = nc.dram_tensor("out_mse", [D, 1], F32, kind="ExternalOutput")

    with tile.TileContext(nc) as tc:
        with (
            tc.tile_pool(name="wpool", bufs=1) as wp,
            tc.tile_pool(name="apool", bufs=1) as ap_,
            tc.tile_pool(name="spool", bufs=2) as sp_,
            tc.tile_pool(name="pp", bufs=1, space="PSUM") as pp,
            tc.tile_pool(name="pp2", bufs=2, space="PSUM") as pp2,
        ):
            ones128 = wp.tile([128, 1], F32)
            nc.vector.memset(ones128[:, :], 1.0)
            ones1 = wp.tile([1, 128], F32)
            nc.vector.memset(ones1[:, :], 1.0)
            eps_t = wp.tile([1, 1], F32)
            nc.vector.memset(eps_t[:, :], LN_EPS)

            # ---- load weights & inputs
            w1t = [wp.tile([128, H2], F32, name=f"w1t{k}") for k in range(8)]
            for k in range(8):
                nc.sync.dma_start(w1t[k][:, :], W1[k * 128:(k + 1) * 128, :])
            w2t = [wp.tile([128, H3], F32, name=f"w2t{k}") for k in range(4)]
            for k in range(4):
                nc.sync.dma_start(w2t[k][:, :], W2[k * 128:(k + 1) * 128, :])
            wht = [wp.tile([128, D], F32, name=f"wht{k}") for k in range(2)]
            for k in range(2):
                nc.sync.dma_start(wht[k][:, :], Wh[k * 128:(k + 1) * 128, :])
            g1s = wp.tile([128, 8], F32); nc.sync.dma_start(g1s[:, :], g1c[:, :])
            b1ls = wp.tile([128, 8], F32); nc.sync.dma_start(b1ls[:, :], b1lc[:, :])
            g2s = wp.tile([128, 4], F32); nc.sync.dma_start(g2s[:, :], g2c[:, :])
            b2ls = wp.tile([128, 4], F32); nc.sync.dma_start(b2ls[:, :], b2lc[:, :])
            b1s = wp.tile([128, 4], F32); nc.sync.dma_start(b1s[:, :], b1c[:, :])
            b2s = wp.tile([128, 2], F32); nc.sync.dma_start(b2s[:, :], b2c[:, :])
            bh2s = wp.tile([D, 1], F32); nc.sync.dma_start(bh2s[:, :], bh2[:, :])

            xk = [ap_.tile([128, R], F32, name=f"xk{k}") for k in range(8)]
            for k in range(8):
                nc.sync.dma_start(xk[k][:, :], xT[k * 128:(k + 1) * 128, :])

            def layer_norm(chunks, n_feat, gs, bls, tag):
                """chunks: list of [128,R] SBUF tiles (features on partitions).
                Returns normalized chunks (new tiles)."""
                nk = len(chunks)
                mu_ps = pp.tile([1, R], F32, name=f"mu_ps_{tag}", tag="mups")
                ss_ps = pp.tile([1, R], F32, name=f"ss_ps_{tag}", tag="ssps")
                for k in range(nk):
                    sq = sp_.tile([128, R], F32, name=f"sq_{tag}", tag=f"sq_{tag}")
                    nc.scalar.activation(sq[:, :], chunks[k][:, :], AF.Square)
                    nc.tensor.matmul(mu_ps[:, :], _r32(ones128[:, :]), _r32(chunks[k][:, :]),
                                     start=(k == 0), stop=(k == nk - 1))
                    nc.tensor.matmul(ss_ps[:, :], _r32(ones128[:, :]), _r32(sq[:, :]),
                                     start=(k == 0), stop=(k == nk - 1))
                inv = 1.0 / n_feat
                mu = ap_.tile([1, R], F32, name=f"mu_{tag}")
                nc.vector.tensor_scalar(mu[:, :], mu_ps[:, :], inv, None, OP.mult)
                ms = ap_.tile([1, R], F32, name=f"ms_{tag}")
                nc.vector.tensor_scalar(ms[:, :], ss_ps[:, :], inv, None, OP.mult)
                mu2 = ap_.tile([1, R], F32, name=f"mu2_{tag}")
                nc.vector.tensor_tensor(mu2[:, :], mu[:, :], mu[:, :], OP.mult)
                var = ap_.tile([1, R], F32, name=f"var_{tag}")
                nc.vector.tensor_tensor(var[:, :], ms[:, :], mu2[:, :], OP.subtract)
                # rstd = exp(-0.5*ln(var+eps))   (exp/ln table set)
                lnv = ap_.tile([1, R], F32, name=f"lnv_{tag}")
                nc.scalar.activation(lnv[:, :], var[:, :], AF.Ln, bias=eps_t[0:1, 0:1], scale=1.0)
                rstd = ap_.tile([1, R], F32, name=f"rstd_{tag}")
                nc.scalar.activation(rstd[:, :], lnv[:, :], AF.Exp, bias=0.0, scale=-0.5)
                # broadcast mu, rstd to [128,R] in PSUM
                mu_b = pp.tile([128, R], F32, name=f"mu_b_{tag}", tag="mub")
                nc.tensor.matmul(mu_b[:, :], _r32(ones1[:, :]), _r32(mu[:, :]), start=True, stop=True)
                rstd_b = pp.tile([128, R], F32, name=f"rstd_b_{tag}", tag="rstdb")
                nc.tensor.matmul(rstd_b[:, :], _r32(ones1[:, :]), _r32(rstd[:, :]), start=True, stop=True)
                outs = []
                for k in range(nk):
                    u = sp_.tile([128, R], F32, name=f"u_{tag}", tag=f"u_{tag}")
                    nc.vector.tensor_tensor(u[:, :], chunks[k][:, :], mu_b[:, :], OP.subtract)
                    u2 = sp_.tile([128, R], F32, name=f"u2_{tag}", tag=f"u2_{tag}")
                    nc.vector.tensor_tensor(u2[:, :], u[:, :], rstd_b[:, :], OP.mult)
                    xn = ap_.tile([128, R], F32, name=f"xn_{tag}{k}")
                    nc.scalar.activation(xn[:, :], u2[:, :], AF.Identity,
                                         bias=bls[:, k:k + 1], scale=gs[:, k:k + 1])
                    outs.append(xn)
                return outs

            xn1 = layer_norm(xk, H, g1s, b1ls, "ln1")

            # ---- mm1 + gelu -> h1T chunks [128,R] x4
            h1 = []
            for m in range(4):
                h1_ps = pp2.tile([128, R], F32, name="h1_ps", tag="h1ps")
                for k in range(8):
                    nc.tensor.matmul(h1_ps[:, :], _r32(w1t[k][:, m * 128:(m + 1) * 128]),
                                     _r32(xn1[k][:, :]), start=(k == 0), stop=(k == 7))
                h1m = ap_.tile([128, R], F32, name=f"h1_{m}")
                nc.scalar.activation(h1m[:, :], h1_ps[:, :], AF.Gelu,
                                     bias=b1s[:, m:m + 1], scale=1.0)
                h1.append(h1m)

            xn2 = layer_norm(h1, H2, g2s, b2ls, "ln2")

            # ---- mm2 + gelu -> h2T chunks [128,R] x2
            h2 = []
            for m in range(2):
                h2_ps = pp2.tile([128, R], F32, name="h2_ps", tag="h1ps")
                for k in range(4):
                    nc.tensor.matmul(h2_ps[:, :], _r32(w2t[k][:, m * 128:(m + 1) * 128]),
                                     _r32(xn2[k][:, :]), start=(k == 0), stop=(k == 3))
                h2m = ap_.tile([128, R], F32, name=f"h2_{m}")
                nc.scalar.activation(h2m[:, :], h2_ps[:, :], AF.Gelu,
                                     bias=b2s[:, m:m + 1], scale=1.0)
                h2.append(h2m)

            # ---- head: logitsT = 0.5 + 0.5*tanh((h2T.T@Wh + bh)/2)
            lg_ps = pp.tile([D, R], F32, name="lg_ps", tag="mups")
            for k in range(2):
                nc.tensor.matmul(lg_ps[:, :], _r32(wht[k][:, :]), _r32(h2[k][:, :]),
                                 start=(k == 0), stop=(k == 1))
            th = ap_.tile([D, R], F32, name="th")
            nc.scalar.activation(th[:, :], lg_ps[:, :], AF.Tanh,
                                 bias=bh2s[:, :], scale=0.5)
            logitsT = ap_.tile([D, R], F32, name="logitsT")
            nc.vector.tensor_scalar(logitsT[:, :], th[:, :], 0.5, 0.5, OP.mult, OP.add)
            nc.sync.dma_start(out_logitsT[:, :], logitsT[:, :])

            # ---- mse partial: sum (logits-labels)^2 per d-partition
            labs = ap_.tile([D, R], F32, name="labs")
            nc.sync.dma_start(labs[:, :], labshT[:, :])
            d1 = ap_.tile([D, R], F32, name="d1")
            nc.vector.tensor_tensor(d1[:, :], logitsT[:, :], labs[:, :], OP.subtract)
            msec = ap_.tile([D, 1], F32, name="msec")
            scrm = ap_.tile([D, R], F32, name="scrm")
            nc.vector.scalar_tensor_tensor(scrm[:, :], d1[:, :], 1.0, d1[:, :],
                                           OP.mult, OP.mult, accum_out=msec[:, :])
            nc.sync.dma_start(out_mse[:, :], msec[:, :])
    nc.compile()
    return nc


def _build_loss():
    nc = bacc.Bacc("TRN2", target_bir_lowering=False, debug=False, num_devices=NC)
    Lwin = nc.dram_tensor("Lwin", [D, W_WIN], F32, kind="ExternalInput")
    Pwin = nc.dram_tensor("Pwin", [D, W_WIN], F32, kind="ExternalInput")
    Lcols = nc.dram_tensor("Lcols", [128, 16], F32, kind="ExternalInput")
    Pcols = nc.dram_tensor("Pcols", [128, 16], F32, kind="ExternalInput")
    maskin = nc.dram_tensor("maskin", [128, 256], BF16, kind="ExternalInput")
    out_stats = nc.dram_tensor("out_stats", [1, 8], F32, kind="ExternalOutput")

    with tile.TileContext(nc) as tc:
        with (
            tc.tile_pool(name="cst", bufs=1) as cst,
            tc.tile_pool(name="bc", bufs=2) as bc,
            tc.tile_pool(name="tp", bufs=2) as tp,
            tc.tile_pool(name="ps", bufs=1, space="PSUM") as ps,
            tc.tile_pool(name="dr", bufs=1, space="DRAM") as dr,
        ):
            ones_bf = cst.tile([128, 1], BF16)
            nc.vector.memset(ones_bf[:, :], 1.0)
            ones_f = cst.tile([128, 1], F32)
            nc.vector.memset(ones_f[:, :], 1.0)
            mask_sb = cst.tile([128, 256], BF16)
            nc.sync.dma_start(mask_sb[:, :], maskin[:, :])

            # one-time: exp rows (scoped pool so the SBUF is reclaimed)
            rows_F = dr.tile([D, W_WIN], F32)
            rows_E = dr.tile([D, W_WIN], F32)
            with tc.tile_pool(name="tmp1", bufs=1) as t1:
                lw = t1.tile([D, W_WIN], F32)
                nc.sync.dma_start(lw[:, :], Lwin[:, :])
                pw = t1.tile([D, W_WIN], F32)
                nc.sync.dma_start(pw[:, :], Pwin[:, :])
                fw = t1.tile([D, W_WIN], F32)
                nc.scalar.activation(fw[:, :], lw[:, :], AF.Exp, bias=0.0, scale=SCALE)
                ew = t1.tile([D, W_WIN], F32)
                nc.scalar.activation(ew[:, :], pw[:, :], AF.Exp, bias=0.0, scale=SCALE)
                nc.sync.dma_start(rows_F[:, :], fw[:, :])
                nc.sync.dma_start(rows_E[:, :], ew[:, :])

            lc = cst.tile([128, 16], F32)
            nc.sync.dma_start(lc[:, :], Lcols[:, :])
            pc = cst.tile([128, 16], F32)
            nc.sync.dma_start(pc[:, :], Pcols[:, :])
            finv = cst.tile([128, 16], F32)
            nc.scalar.activation(finv[:, :], lc[:, :], AF.Exp, bias=0.0, scale=-SCALE)
            einv = cst.tile([128, 16], F32)
            nc.scalar.activation(einv[:, :], pc[:, :], AF.Exp, bias=0.0, scale=-SCALE)

            racc = cst.tile([128, 16], F32)
            gpc = cst.tile([128, 16], F32)
            mspc = cst.tile([128, 16], F32)
            cnt_ps = ps.tile([1, 512], F32)

            first = [True]
            slices = [(0, 512), (512, 1024), (1024, 1536), (1536, 2048), (2048, 2176)]
            for d in range(D):
                Fb = bc.tile([128, W_WIN], F32, name="Fb", tag="Fb")
                nc.sync.dma_start(Fb[:, :], rows_F[d:d + 1, :].partition_broadcast(128))
                Eb = bc.tile([128, W_WIN], F32, name="Eb", tag="Eb")
                nc.sync.dma_start(Eb[:, :], rows_E[d:d + 1, :].partition_broadcast(128))
                Pb = bc.tile([128, W_WIN], F32, name="Pb", tag="Pb")
                nc.sync.dma_start(Pb[:, :], Pwin[d:d + 1, :].partition_broadcast(128))
                for s in range(4):
                    col = s * 4 + d
                    j0 = 128 * s
                    spl = tp.tile([128, W_UNIT], F32, name="spl", tag="spl")
                    nc.scalar.activation(spl[:, :], Fb[:, j0:j0 + W_UNIT], AF.Ln,
                                         bias=1.0, scale=finv[:, col:col + 1])
                    tt_ = tp.tile([128, W_UNIT], F32, name="tt_", tag="tt_")
                    nc.scalar.activation(tt_[:, :], spl[:, :], AF.Exp, bias=0.0, scale=-1.0)
                    sp = tp.tile([128, W_UNIT], F32, name="sp", tag="sp")
                    nc.scalar.activation(sp[:, :], Eb[:, j0:j0 + W_UNIT], AF.Ln,
                                         bias=1.0, scale=einv[:, col:col + 1])
                    m1 = tp.tile([128, W_UNIT], BF16, name="m1", tag="m1")
                    nc.vector.tensor_scalar(m1[:, :], spl[:, :], float(HI), None, OP.is_gt)
                    m = tp.tile([128, W_UNIT], BF16, name="m", tag="m")
                    nc.vector.scalar_tensor_tensor(m[:, :], spl[:, :], float(LO), m1[:, :],
                                                   OP.is_lt, OP.add)
                    # diag + wrap masks
                    nc.vector.tensor_tensor(m[:, 0:128], m[:, 0:128],
                                            mask_sb[:, 0:128], OP.mult)
                    nc.vector.tensor_tensor(m[:, 2048:2176], m[:, 2048:2176],
                                            mask_sb[:, 128:256], OP.mult)
                    gt = tp.tile([128, W_UNIT], F32, name="gt", tag="gt")
                    nc.vector.scalar_tensor_tensor(gt[:, :], tt_[:, :], 1.0, m[:, :],
                                                   OP.subtract, OP.mult,
                                                   accum_out=racc[:, col:col + 1])
                    scr = tp.tile([128, W_UNIT], F32, name="scr", tag="scr", bufs=1)
                    nc.vector.scalar_tensor_tensor(scr[:, :], gt[:, :], 1.0,
                                                   Pb[:, j0:j0 + W_UNIT],
                                                   OP.mult, OP.mult,
                                                   accum_out=gpc[:, col:col + 1])
                    msp = tp.tile([128, W_UNIT], F32, name="msp", tag="msp", bufs=1)
                    nc.vector.scalar_tensor_tensor(msp[:, :], sp[:, :], 1.0, m[:, :],
                                                   OP.mult, OP.mult,
                                                   accum_out=mspc[:, col:col + 1])
                    for (a, b_) in slices:
                        w = b_ - a
                        nc.tensor.matmul(cnt_ps[:, 0:w], ones_bf[:, :], m[:, a:b_],
                                         start=first[0], stop=False)
                        first[0] = False

            # ---- finals
            cnt_sb = cst.tile([1, 512], F32)
            nc.vector.tensor_copy(cnt_sb[:, :], cnt_ps[:, :])
            stats = cst.tile([1, 8], F32)
            nc.vector.memset(stats[:, :], 0.0)
            nc.vector.tensor_reduce(stats[0:1, 0:1], cnt_sb[:, :],
                                    mybir.AxisListType.X, OP.add)
            rpp = cst.tile([128, 16], F32)
            nc.vector.tensor_tensor(rpp[:, :], racc[:, :], pc[:, :], OP.mult)
            stack = cst.tile([128, 3], F32)
            nc.vector.tensor_reduce(stack[:, 0:1], rpp[:, :], mybir.AxisListType.X, OP.add)
            nc.vector.tensor_reduce(stack[:, 1:2], gpc[:, :], mybir.AxisListType.X, OP.add)
            nc.vector.tensor_reduce(stack[:, 2:3], mspc[:, :], mybir.AxisListType.X, OP.add)
            fin_ps = ps.tile([1, 4], F32)
            nc.tensor.matmul(fin_ps[:, 0:3], ones_f[:, :], stack[:, :], start=True, stop=True)
            nc.vector.tensor_copy(stats[0:1, 1:2], fin_ps[:, 2:3])
            nc.vector.tensor_copy(stats[0:1, 2:4], fin_ps[:, 0:2])
            nc.sync.dma_start(out_stats[:, :], stats[:, :])
    nc.compile()
    return nc


def _get(name):
    if name not in _cache:
        _cache[name] = _build_mlp() if name == "mlp" else _build_loss()
    return _cache[name]


def kernel(pooled, labels, ln1_g, ln1_b, W1, b1, ln2_g, ln2_b, W2, b2, Wh, bh):
    pooled = np.asarray(pooled, np.float32)
    labels = np.asarray(labels, np.float32)
    W1 = np.asarray(W1, np.float32); W2 = np.asarray(W2, np.float32)
    Wh = np.asarray(Wh, np.float32)

    # ---- kernel A: MLP ----
    pooledT = np.ascontiguousarray(pooled.T)              # [H, B]
    labelsT = np.ascontiguousarray(labels.T)              # [D, B]
    common = {
        "W1": W1, "W2": W2, "Wh": Wh,
        "b1c": np.ascontiguousarray(np.asarray(b1, np.float32).reshape(4, 128).T),
        "b2c": np.ascontiguousarray(np.asarray(b2, np.float32).reshape(2, 128).T),
        "bh2": np.asarray(bh, np.float32).reshape(D, 1) * 0.5,
        "g1c": np.ascontiguousarray(np.asarray(ln1_g, np.float32).reshape(8, 128).T),
        "b1lc": np.ascontiguousarray(np.asarray(ln1_b, np.float32).reshape(8, 128).T),
        "g2c": np.ascontiguousarray(np.asarray(ln2_g, np.float32).reshape(4, 128).T),
        "b2lc": np.ascontiguousarray(np.asarray(ln2_b, np.float32).reshape(4, 128).T),
    }
    in_maps_a = []
    for c in range(NC):
        in_maps_a.append(dict(
            common,
            xT=np.ascontiguousarray(pooledT[:, c * R:(c + 1) * R]),
            labshT=np.ascontiguousarray(labelsT[:, c * R:(c + 1) * R]),
        ))
    nc_a = _get("mlp")
    res_a = bass_utils.run_bass_kernel_spmd(
        nc_a, in_maps_a, core_ids=list(range(NC)),
        trace=bool(int(os.environ.get("KTRACE", "0"))))
    logits = np.concatenate(
        [res_a.results[c]["out_logitsT"].T for c in range(NC)], 0)  # [B, D]
    mse_ss = sum(float(res_a.results[c]["out_mse"].sum()) for c in range(NC))

    # ---- kernel B: pairwise loss ----
    logitsT = np.ascontiguousarray(logits.T)              # [D, B]
    Pext = np.concatenate([logitsT, logitsT[:, :W_WIN]], 1)
    Lext = np.concatenate([labelsT, labelsT[:, :W_WIN]], 1)
    tri = np.triu(np.ones((128, 128), np.float32), 1)
    low = 1.0 - tri                                        # lower incl diag
    import ml_dtypes
    in_maps_b = []
    for c in range(NC):
        wrap = tri if c < 4 else low
        maskin = np.concatenate([tri, wrap], 1).astype(ml_dtypes.bfloat16)
        pcols = np.empty((128, 16), np.float32)
        lcols = np.empty((128, 16), np.float32)
        for s in range(4):
            for d in range(D):
                pcols[:, s * 4 + d] = logits[c * R + 128 * s: c * R + 128 * (s + 1), d]
                lcols[:, s * 4 + d] = labels[c * R + 128 * s: c * R + 128 * (s + 1), d]
        in_maps_b.append({
            "Lwin": np.ascontiguousarray(Lext[:, c * R: c * R + W_WIN]),
            "Pwin": np.ascontiguousarray(Pext[:, c * R: c * R + W_WIN]),
            "Lcols": lcols, "Pcols": pcols, "maskin": maskin,
        })
    nc_b = _get("loss")
    res_b = bass_utils.run_bass_kernel_spmd(
        nc_b, in_maps_b, core_ids=list(range(NC)),
        trace=bool(int(os.environ.get("KTRACE", "0"))))

    cnt = 0.0; spsum = 0.0; rp = 0.0; gp = 0.0
    for c in range(NC):
        st = np.asarray(res_b.results[c]["out_stats"], np.float64)
        cnt += st[0, 0]; spsum += st[0, 1]; rp += st[0, 2]; gp += st[0, 3]

    ordering_sum = spsum + SCALE * (gp - rp)
    ordering = ordering_sum / cnt if cnt > 0 else 0.0
    mse = mse_ss / (B * D)
    loss = np.float32(0.5 * mse + 0.5 * ordering)
    kernel._last = (res_a, res_b)
    return np.array(loss, np.float32), logits
